# revision 18
# baseline (speedup 1.0000x reference)
"""Trainium2 Bass kernel for nn_AwesomeGRU (SEQ=512, B=64, DIM=1024, UNITS=1024).

Algorithm: the `reset` input zeroes h *before* each masked step, so each batch
row's recurrence splits into independent segments (h carries over only within
a segment). Classic packed-sequence reformulation:

  host: enumerate segments, sort by length desc, deal round-robin to 8 cores,
        lay tokens out depth-major ((depth, segment-rank) order). Pass j
        processes all tokens at depth j — a contiguous row block whose h
        inputs are a PREFIX of pass j-1's outputs (no gather).
  core: for each pass j: PSUM <- x_j @ W_ih^T (+ h_j @ W_hh^T if j>0), then
        gates elementwise, h_out -> DRAM (it IS the output) + fp16 copy in
        SBUF for pass j+1's matmul.
  host: inverse-permute output tokens to (seq, b, units).

Everything is feature-major on device: activations stored (units, rows) so
no transposes are ever needed. All weights are pre-scaled x64 host-side
(lossless exponent shift) so W_hh fits fp8e4m3 normals; every PSUM read
applies scale=1/64. Matmul dtype mix:
  - gi (x-projection): fp16 x fp16 (fp8 fails the 2e-2 gate: ~3e-2).
  - gh r,z gates, front passes: DoubleRow fp8 (2x PE throughput at FD>=128;
    h is cast to an fp8 [K,2,m] interleaved copy each pass).
  - gh n gate: fp16 (tanh passes h-quant error straight through; keeping it
    fp16 halves the total error for 1/3 the cost).
  - tail passes (m<=64): LDW/dispatch-bound, DoubleRow loses; r,z use plain
    fp8-stationary weights (FWL) with fp16 moving h; gates are batched
    across all 8 unit-groups in one op ([P, 8, m] APs over quarter-banks)
    with the n bias injected by a rank-1 K=1 matmul.

Self-contained: derives everything from the runtime value of `reset`.
"""
import os
import numpy as np
import ml_dtypes

import concourse.bacc as bacc
import concourse.mybir as mybir
import concourse.tile as tile
from concourse.bass_utils import run_bass_kernel_spmd

SEQ, B, DIM, UNITS = 512, 64, 1024, 1024
NCORES = 8
P = 128
CG = DIM // P        # 8 contraction groups per matmul side
C2 = CG // 2         # 4 DoubleRow contraction groups (K=256 each)
UG = UNITS // P      # 8 unit groups
CH = 512             # row-chunk (free dim / PSUM bank)
WS = 64.0            # weight pre-scale (exact in fp16/fp8 exponents)
dt = mybir.dt
f32 = dt.float32
f16 = dt.float16
e4 = dt.float8e4
DR = mybir.MatmulPerfMode.DoubleRow

LAST_EXEC_NS = None  # set when GRU_TRACE=1


def _c16(n):
    return (n + 15) // 16 * 16


# ---------------------------------------------------------------- host plan

def _build_plan(reset_sb, h0_any):
    """reset_sb: (SEQ, B) bool. Returns (m_j schedule, per-core token maps).

    Segment starts: t=0 always (h0 seed row: h0[b] unless reset[0,b]), and
    every t>0 with reset=1 (h zeroed exactly).
    """
    segs = []  # (length, b, t_start)
    for b in range(B):
        col = reset_sb[:, b]
        starts = [0] + [t for t in range(1, SEQ) if col[t]]
        for i, s in enumerate(starts):
            e = starts[i + 1] if i + 1 < len(starts) else SEQ
            segs.append((e - s, b, s))
    segs.sort(key=lambda x: (-x[0], x[1], x[2]))
    Lmax = segs[0][0]
    n_j = [0] * Lmax
    for L, _, _ in segs:
        for j in range(L):
            n_j[j] += 1
    m_j = [(n + NCORES - 1) // NCORES for n in n_j]

    plans = []
    for c in range(NCORES):
        mysegs = segs[c::NCORES]
        tok = np.full(sum(m_j), -1, np.int64)  # flat t*B+b index or -1 pad
        seed_b = np.full(m_j[0], -1, np.int64)  # batch row for h seed (pass 0)
        off = 0
        for j in range(Lmax):
            for r in range(m_j[j]):
                if r < len(mysegs) and mysegs[r][0] > j:
                    L, b, s = mysegs[r]
                    tok[off + r] = (s + j) * B + b
                    if j == 0 and s == 0 and h0_any and not reset_sb[0, b]:
                        seed_b[r] = b
            off += m_j[j]
        plans.append((tok, seed_b))
    return m_j, plans


# ------------------------------------------------------------- device build

def _chunks(m):
    """Split m rows into balanced chunks of <= CH with 16-aligned offsets."""
    nch = (m + CH - 1) // CH
    step = min(CH, _c16((m + nch - 1) // nch))
    out, off = [], 0
    while off < m:
        f = min(step, m - off)
        out.append((off, f))
        off += f
    return out


def _build_nc(m_j, use_seed, j_pre, j_tail):
    """j_pre: first pass whose gi comes from the fp16 presweep buffer.
    j_tail: first pass using the batched small-m tail schema (m_j <= 64)."""
    Lmax = len(m_j)
    N_pad = sum(m_j)
    M_off = np.cumsum([0] + m_j)  # row offset of each pass block
    R0 = int(M_off[j_pre]) if j_pre < Lmax else N_pad  # presweep row range
    RN = N_pad - R0

    nc = bacc.Bacc("TRN2", target_bir_lowering=False, debug=False,
                   num_devices=NCORES)
    xT = nc.dram_tensor("xT", [DIM, N_pad], f16, kind="ExternalInput")
    wihT = nc.dram_tensor("wihT", [DIM, 3 * UNITS], f16, kind="ExternalInput")
    whhnT = nc.dram_tensor("whhnT", [UNITS, UNITS], f16, kind="ExternalInput")
    whh8T = nc.dram_tensor("whh8T", [UNITS, 2 * UNITS], e4, kind="ExternalInput")
    biases = nc.dram_tensor("biases", [UNITS, 4], f32, kind="ExternalInput")
    # rank-1 bias rows (x64): [b_sum_r, b_sum_z, b_ihn, b_hhn]
    bnrow = nc.dram_tensor("bnrow", [4, UNITS], f16, kind="ExternalInput")
    # quarter-bank selector: sel[k, col] = (col // 64 == k)
    bsel = nc.dram_tensor("bsel", [UG, CH], f16, kind="ExternalInput")
    # 64*b_hhn as [8, 128]: row k = units k*128..(k+1)*128
    bn8 = nc.dram_tensor("bn8", [UG, P], f16, kind="ExternalInput")
    outT = nc.dram_tensor("outT", [UNITS, N_pad], f32, kind="ExternalOutput")
    hseedT = None
    if use_seed:
        hseedT = nc.dram_tensor("hseedT", [UNITS, m_j[0]], f16,
                                kind="ExternalInput")

    Sig = mybir.ActivationFunctionType.Sigmoid
    Tanh = mybir.ActivationFunctionType.Tanh
    Copy = mybir.ActivationFunctionType.Copy
    ADD = mybir.AluOpType.add
    MULT = mybir.AluOpType.mult
    IS = 1.0 / WS

    with tile.TileContext(nc) as tc:
        with (
            tc.tile_pool(name="wpool", bufs=1) as wpool,
            tc.tile_pool(name="xpool", bufs=2) as xpool,
            tc.tile_pool(name="hpool", bufs=2) as hpool,
            tc.tile_pool(name="spool", bufs=2) as spool,
            tc.tile_pool(name="ppool", bufs=2, space="PSUM") as ppool,
        ):
            wih_t = wpool.tile([P, CG, 3 * UNITS], f16, tag="wih")
            whhn_t = wpool.tile([P, CG, UNITS], f16, tag="whhn")
            whh8_t = wpool.tile([P, C2, 2, 2 * UNITS], e4, tag="whh8")
            ones_t = wpool.tile([1, CH], f16, tag="ones")
            bn_t = wpool.tile([1, 4, UG, P], f16, tag="bnrow")
            zrow_t = wpool.tile([1, P], f16, tag="zrow")
            sel_t = wpool.tile([UG, CH], f16, tag="bsel")
            bn8_t = wpool.tile([UG, P], f16, tag="bn8")

            x_tiles = {}

            def get_x_tile(jj, ooff, ff):
                key = (jj, ooff)
                if key not in x_tiles:
                    x_t = xpool.tile([P, CG, CH], f16, tag="x", name="x_t")
                    bb = int(M_off[jj]) + ooff
                    for c in range(CG):
                        nc.sync.dma_start(out=x_t[:, c, :ff],
                                          in_=xT[c * P:(c + 1) * P, bb: bb + ff])
                    x_tiles[key] = x_t
                return x_tiles[key]

            # DMA emission order = need order: r-gate weights, first x chunk,
            # remaining W_ih gates + biases, second x chunk. W_hh and the
            # presweep are emitted later (needed from pass 1 / pass j_pre).
            for c in range(CG):
                nc.sync.dma_start(out=wih_t[:, c, 0:UNITS],
                                  in_=wihT[c * P:(c + 1) * P, 0:UNITS])
            ch0 = _chunks(m_j[0])
            get_x_tile(0, *ch0[0])
            for g in (1, 2):
                for c in range(CG):
                    nc.sync.dma_start(
                        out=wih_t[:, c, g * UNITS:(g + 1) * UNITS],
                        in_=wihT[c * P:(c + 1) * P, g * UNITS:(g + 1) * UNITS])
            b_t = wpool.tile([P, UG, 4], f32, tag="bias")
            for g in range(UG):
                nc.sync.dma_start(out=b_t[:, g, :], in_=biases[g * P:(g + 1) * P, :])
            nc.vector.memset(ones_t[:, :], 1.0)
            for (oo, ff) in ch0[1:]:
                get_x_tile(0, oo, ff)

            def emit_whh():
                # r,z DoubleRow-interleaved fp8 tiles: slot [c2, i] holds
                # contraction features (2*c2+i)*128 .. +128.
                for c2 in range(C2):
                    for i in range(2):
                        r0 = (2 * c2 + i) * P
                        nc.sync.dma_start(out=whh8_t[:, c2, i, :],
                                          in_=whh8T[r0:r0 + P, :])
                for c in range(CG):
                    nc.sync.dma_start(out=whhn_t[:, c, :],
                                      in_=whhnT[c * P:(c + 1) * P, :])
                nc.sync.dma_start(out=bn_t[:, :, :, :], in_=bnrow[:, :])
                nc.sync.dma_start(out=sel_t[:, :], in_=bsel[:, :])
                nc.sync.dma_start(out=bn8_t[:, :], in_=bn8[:, :])
                nc.vector.memset(zrow_t[:, :], 0.0)

            gi_pre = (wpool.tile([P, 3 * UG, RN], f16, tag="gi_pre",
                                 name="gi_pre")
                      if RN > 0 else None)

            def emit_presweep():
                # gi for all deep-pass rows in one efficient batched matmul.
                # gi_pre holds 64*(gi + bias): r,z biased b_ih+b_hh, n b_ih;
                # the bias arrives via a rank-1 (K=1) seed matmul per tile.
                with nc.named_scope("presweep"):
                    xp_t = xpool.tile([P, CG, CH], f16, tag="x", name="xp_t")
                    for c in range(CG):
                        nc.sync.dma_start(out=xp_t[:, c, :RN],
                                          in_=xT[c * P:(c + 1) * P, R0:N_pad])
                    for gu in range(3 * UG):
                        g, u = divmod(gu, UG)
                        ps_p = ppool.tile([P, CH], f32, tag="ps_gin",
                                          name="ps_pre")
                        nc.tensor.matmul(ps_p[:, :RN], lhsT=bn_t[:, g, u, :],
                                         rhs=ones_t[:, :RN], start=True,
                                         stop=False, skip_group_check=True)
                        for c in range(CG):
                            nc.tensor.matmul(
                                ps_p[:, :RN],
                                lhsT=wih_t[:, c, gu * P:(gu + 1) * P],
                                rhs=xp_t[:, c, :RN],
                                start=False, stop=(c == CG - 1),
                                skip_group_check=True)
                        nc.vector.tensor_copy(gi_pre[:, gu, :], ps_p[:, :RN])

            if use_seed:
                emit_whh()  # pass 0 already needs W_hh

            h_cur = None   # fp16 SBUF (P, CG, m_j[j]) input h for current pass
            h8_cur = None  # fp8 DR-interleaved (P, C2, 2, mpad) same h
            for j in range(Lmax):
                if j == j_pre and gi_pre is not None:
                    emit_presweep()
                scope = nc.named_scope(f"pass{j:02d}")
                scope.__enter__()
                m = m_j[j]
                m_next = m_j[j + 1] if j + 1 < Lmax else 0
                has_h = (j > 0) or use_seed
                pre = j >= j_pre
                tail = j >= j_tail
                base = int(M_off[j])
                h_next = (hpool.tile([P, CG, m_next], f16, tag="hbuf",
                                     name=f"hbuf{j}")
                          if m_next > 0 else None)
                h8_next = (hpool.tile([P, C2, 2, _c16(m_next)], e4, tag="h8buf",
                                      name=f"h8buf{j}")
                           if (m_next > 0 and j + 1 < j_tail) else None)

                if tail:
                    _emit_tail_pass(nc, ppool, spool, j, m, m_next, base, R0,
                                    whh8_t, whhn_t, zrow_t, sel_t, bn8_t,
                                    ones_t, gi_pre, h_cur, h_next, outT,
                                    Sig, Tanh, ADD, MULT, IS)
                    h_cur = h_next
                    h8_cur = None
                    scope.__exit__(None, None, None)
                    continue

                for ci, (off, f) in enumerate(_chunks(m)):
                    if not pre:
                        x_t = get_x_tile(j, off, f)
                    if j == 0 and use_seed:
                        hs_t = xpool.tile([P, CG, CH], f16, tag="hseed",
                                          name="hs_t", bufs=1)
                        hs8_t = xpool.tile([P, C2, 2, CH], e4, tag="hseed8",
                                           name="hs8_t", bufs=1)
                        for c in range(CG):
                            nc.sync.dma_start(
                                out=hs_t[:, c, :f],
                                in_=hseedT[c * P:(c + 1) * P, off: off + f])
                        for c in range(CG):
                            nc.vector.tensor_copy(
                                hs8_t[:, c // 2, c % 2, :f], hs_t[:, c, :f])
                        h_in = lambda c: hs_t[:, c, :f]
                        h8_in = lambda c2: hs8_t[:, c2, :, :f]
                    elif has_h:
                        h_in = lambda c: h_cur[:, c, off: off + f]
                        h8_in = lambda c2: h8_cur[:, c2, :, off: off + f]
                    else:
                        h_in = None
                        h8_in = None
                    # presweep-relative row slice for this chunk
                    p0 = base + off - R0

                    def x_mms(ps, gate, stop_at_end):
                        for c in range(CG):
                            nc.tensor.matmul(
                                ps[:, :f],
                                lhsT=wih_t[:, c, gate * UNITS + u * P:
                                           gate * UNITS + (u + 1) * P],
                                rhs=x_t[:, c, :f],
                                start=(c == 0),
                                stop=(stop_at_end and c == CG - 1))

                    def hrz_mms(ps, gate, c2s, do_start, do_stop):
                        # DoubleRow fp8: K=256 per matmul
                        c2s = list(c2s)
                        if os.environ.get("GRU_NODR", "0") == "1":
                            for c2 in c2s:
                                for i in range(2):
                                    nc.tensor.matmul(
                                        ps[:, :f],
                                        lhsT=whh8_t[:, c2, i,
                                                    gate * UNITS + u * P:
                                                    gate * UNITS + (u + 1) * P],
                                        rhs=h_in(2 * c2 + i),
                                        start=(do_start and c2 == c2s[0] and i == 0),
                                        stop=(do_stop and c2 == c2s[-1] and i == 1),
                                        skip_group_check=True)
                            return
                        for c2 in c2s:
                            nc.tensor.matmul(
                                ps[:, :f],
                                lhsT=whh8_t[:, c2, :, gate * UNITS + u * P:
                                            gate * UNITS + (u + 1) * P],
                                rhs=h8_in(c2),
                                start=(do_start and c2 == c2s[0]),
                                stop=(do_stop and c2 == c2s[-1]),
                                perf_mode=DR,
                                skip_group_check=True)

                    def hn_mms(ps, cs, do_start, do_stop):
                        cs = list(cs)
                        for c in cs:
                            nc.tensor.matmul(
                                ps[:, :f],
                                lhsT=whhn_t[:, c, u * P:(u + 1) * P],
                                rhs=h_in(c),
                                start=(do_start and c == cs[0]),
                                stop=(do_stop and c == cs[-1]),
                                skip_group_check=True)

                    for u in range(UG):
                        ps_r = ppool.tile([P, CH], f32, tag="ps_r")
                        ps_z = ppool.tile([P, CH], f32, tag="ps_z")
                        if not pre:
                            ps_gin = ppool.tile([P, CH], f32, tag="ps_gin")
                        ps_ghn = (ppool.tile([P, CH], f32, tag="ps_ghn",
                                             name="ps_ghn")
                                  if has_h else None)

                        # For the first unit-tile of a chunk, defer every
                        # gate's last h-matmul to the end: it waits on the
                        # previous pass's last h cast, and deferring lets the
                        # other matmuls run during that wait.
                        split = has_h and u == 0 and off == 0
                        early2 = range(C2 - 1) if split else range(C2)
                        early = range(CG - 1) if split else range(CG)
                        if not pre:
                            x_mms(ps_r, 0, stop_at_end=not has_h)
                            if has_h:
                                hrz_mms(ps_r, 0, early2, False, not split)
                            x_mms(ps_z, 1, stop_at_end=not has_h)
                            if has_h:
                                hrz_mms(ps_z, 1, early2, False, not split)
                            x_mms(ps_gin, 2, stop_at_end=True)
                            if has_h:
                                hn_mms(ps_ghn, early, True, not split)
                        else:
                            hrz_mms(ps_r, 0, early2, True, not split)
                            hrz_mms(ps_z, 1, early2, True, not split)
                            hn_mms(ps_ghn, early, True, not split)
                        if split:
                            hrz_mms(ps_r, 0, [C2 - 1], False, True)
                            hrz_mms(ps_z, 1, [C2 - 1], False, True)
                            hn_mms(ps_ghn, [CG - 1], False, True)

                        r_sb = spool.tile([P, CH], f32, tag="r")
                        z_sb = spool.tile([P, CH], f32, tag="z")
                        n_sb = spool.tile([P, CH], f32, tag="n")
                        h_sb = spool.tile([P, CH], f32, tag="r" if use_seed else "h",
                                          name="h_sb")
                        t2 = spool.tile([P, CH], f32, tag="t2")
                        if pre:
                            # gi_pre is x64-scaled and pre-biased
                            nc.vector.tensor_add(r_sb[:, :f], ps_r[:, :f],
                                                 gi_pre[:, u, p0:p0 + f])
                            nc.scalar.activation(r_sb[:, :f], r_sb[:, :f], Sig,
                                                 scale=IS)
                            nc.vector.tensor_add(z_sb[:, :f], ps_z[:, :f],
                                                 gi_pre[:, UG + u, p0:p0 + f])
                            nc.scalar.activation(z_sb[:, :f], z_sb[:, :f], Sig,
                                                 scale=IS)
                            # t2 = (ps_ghn + 64 b_hhn) * r   (still x64)
                            nc.vector.scalar_tensor_tensor(
                                t2[:, :f], ps_ghn[:, :f], b_t[:, u, 3:4],
                                r_sb[:, :f], op0=ADD, op1=MULT)
                            arg = spool.tile([P, CH], f32, tag="d", name="arg")
                            nc.vector.tensor_add(
                                arg[:, :f], t2[:, :f],
                                gi_pre[:, 2 * UG + u, p0:p0 + f])
                            nc.scalar.activation(n_sb[:, :f], arg[:, :f], Tanh,
                                                 scale=IS)
                        else:
                            nc.scalar.activation(r_sb[:, :f], ps_r[:, :f], Sig,
                                                 bias=b_t[:, u, 0:1], scale=IS)
                            nc.scalar.activation(z_sb[:, :f], ps_z[:, :f], Sig,
                                                 bias=b_t[:, u, 1:2], scale=IS)
                            if has_h:
                                # t2 = (ps_ghn + 64 b_hhn) * r
                                nc.vector.scalar_tensor_tensor(
                                    t2[:, :f], ps_ghn[:, :f], b_t[:, u, 3:4],
                                    r_sb[:, :f], op0=ADD, op1=MULT)
                                arg = spool.tile([P, CH], f32, tag="d", name="arg")
                                nc.vector.tensor_add(arg[:, :f], t2[:, :f],
                                                     ps_gin[:, :f])
                                nc.scalar.activation(n_sb[:, :f], arg[:, :f],
                                                     Tanh, bias=b_t[:, u, 2:3],
                                                     scale=IS)
                            else:
                                # t2 = r*(64 b_hhn) + ps_gin ; n = tanh(t2/64 + b_ihn)
                                nc.vector.scalar_tensor_tensor(
                                    t2[:, :f], r_sb[:, :f], b_t[:, u, 3:4],
                                    ps_gin[:, :f], op0=MULT, op1=ADD)
                                nc.scalar.activation(n_sb[:, :f], t2[:, :f],
                                                     Tanh, bias=b_t[:, u, 2:3],
                                                     scale=IS)
                        if has_h:
                            # h = n + z*(h_prev - n)   (h_prev via fp16 tile)
                            d_sb = spool.tile([P, CH], f32, tag="d")
                            nc.vector.tensor_sub(d_sb[:, :f], h_in(u), n_sb[:, :f])
                            zd = spool.tile([P, CH], f32, tag="t2", name="zd")
                            nc.vector.tensor_mul(zd[:, :f], z_sb[:, :f], d_sb[:, :f])
                            nc.vector.tensor_add(h_sb[:, :f], n_sb[:, :f], zd[:, :f])
                        else:
                            # h = (1-z)*n = n - z*n
                            zd = spool.tile([P, CH], f32, tag="t2", name="zd")
                            nc.vector.tensor_mul(zd[:, :f], z_sb[:, :f], n_sb[:, :f])
                            nc.vector.tensor_sub(h_sb[:, :f], n_sb[:, :f], zd[:, :f])

                        nc.sync.dma_start(
                            out=outT[u * P:(u + 1) * P, base + off: base + off + f],
                            in_=h_sb[:, :f])
                        pf = min(m_next - off, f)
                        if pf > 0:
                            nc.vector.tensor_copy(h_next[:, u, off: off + pf],
                                                  h_sb[:, :pf])
                            if h8_next is not None:
                                nc.vector.tensor_copy(
                                    h8_next[:, u // 2, u % 2, off: off + pf],
                                    h_sb[:, :pf])
                    if j == 0 and ci == 0 and not use_seed:
                        emit_whh()  # W_hh drains during pass-0 compute
                    if not pre and (j, off) in x_tiles:
                        del x_tiles[(j, off)]  # consumed; let the slot recycle
                h_cur = h_next
                h8_cur = h8_next
                scope.__exit__(None, None, None)
    nc.compile()
    return nc


def _emit_tail_pass(nc, ppool, spool, j, m, m_next, base, R0, whh8_t, whhn_t,
                    zrow_t, sel_t, bn8_t, ones_t, gi_pre, h_cur, h_next, outT,
                    Sig, Tanh, ADD, MULT, IS):
    """Small-m pass (m <= 64): all 8 unit-groups packed into quarter-banks,
    gates batched across u in single [P, 8, m] ops.

    PSUM rule learned the hard way: only ONE start=True accumulation group
    may be opened per bank — a later start corrupts earlier quarter-groups.
    So each bank is seeded by a single full-bank matmul (zeros for r/z, the
    K=8 selector matmul writing b_hhn per quarter for n), and every quarter
    matmul accumulates with start=False."""
    UGQ = 64  # per-u column budget inside the [P, 8, 64] psum view
    f = m
    p0 = base - R0
    ps_r = ppool.tile([P, UG, UGQ], mybir.dt.float32, tag="ps_r", name="tps_r")
    ps_z = ppool.tile([P, UG, UGQ], mybir.dt.float32, tag="ps_z", name="tps_z")
    ps_n = ppool.tile([P, UG, UGQ], mybir.dt.float32, tag="ps_ghn", name="tps_n")

    # Bank seeds (no h dependency: the PE runs them while the previous
    # pass's gates finish): r/z zeroed, n gets 64*b_hhn[col//64-slice].
    nc.tensor.matmul(ps_r[:, :, :], lhsT=zrow_t[:, :], rhs=ones_t[:, :],
                     start=True, stop=True, skip_group_check=True)
    nc.tensor.matmul(ps_z[:, :, :], lhsT=zrow_t[:, :], rhs=ones_t[:, :],
                     start=True, stop=True, skip_group_check=True)
    nc.tensor.matmul(ps_n[:, :, :], lhsT=bn8_t[:, :], rhs=sel_t[:, :],
                     start=True, stop=True, skip_group_check=True)
    for gate, ps in ((0, ps_r), (1, ps_z)):
        for u in range(UG):
            for c in range(CG):
                nc.tensor.matmul(
                    ps[:, u, :f],
                    lhsT=whh8_t[:, c // 2, c % 2,
                                gate * UNITS + u * P:gate * UNITS + (u + 1) * P],
                    rhs=h_cur[:, c, :f],
                    start=False, stop=(c == CG - 1), skip_group_check=True)
    for u in range(UG):
        for c in range(CG):
            nc.tensor.matmul(
                ps_n[:, u, :f],
                lhsT=whhn_t[:, c, u * P:(u + 1) * P],
                rhs=h_cur[:, c, :f],
                start=False, stop=(c == CG - 1), skip_group_check=True)

    r_sb = spool.tile([P, UG, UGQ], mybir.dt.float32, tag="r", name="ttr")
    z_sb = spool.tile([P, UG, UGQ], mybir.dt.float32, tag="z", name="ttz")
    n_sb = spool.tile([P, UG, UGQ], mybir.dt.float32, tag="n", name="ttn")
    h_sb = spool.tile([P, UG, UGQ], mybir.dt.float32, tag="h", name="tth")
    t2 = spool.tile([P, UG, UGQ], mybir.dt.float32, tag="t2", name="ttt2")
    d_sb = spool.tile([P, UG, UGQ], mybir.dt.float32, tag="d", name="ttd")

    # gi_pre slices are x64-scaled and pre-biased
    gr = gi_pre[:, 0:UG, p0:p0 + f]
    gz = gi_pre[:, UG:2 * UG, p0:p0 + f]
    gn = gi_pre[:, 2 * UG:3 * UG, p0:p0 + f]
    nc.vector.tensor_add(r_sb[:, :, :f], ps_r[:, :, :f], gr)
    nc.scalar.activation(r_sb[:, :, :f], r_sb[:, :, :f], Sig, scale=IS)
    nc.vector.tensor_add(z_sb[:, :, :f], ps_z[:, :, :f], gz)
    nc.scalar.activation(z_sb[:, :, :f], z_sb[:, :, :f], Sig, scale=IS)
    # n = tanh(r*(ghn + b_hhn) + gi_n + b_ihn): ps_n holds 64*(ghn + b_hhn)
    nc.vector.tensor_mul(t2[:, :, :f], r_sb[:, :, :f], ps_n[:, :, :f])
    nc.vector.tensor_add(n_sb[:, :, :f], t2[:, :, :f], gn)
    nc.scalar.activation(n_sb[:, :, :f], n_sb[:, :, :f], Tanh, scale=IS)
    # h = n + z*(h_prev - n)
    nc.vector.tensor_sub(d_sb[:, :, :f], h_cur[:, :, :f], n_sb[:, :, :f])
    nc.vector.tensor_mul(t2[:, :, :f], z_sb[:, :, :f], d_sb[:, :, :f])
    nc.vector.tensor_add(h_sb[:, :, :f], n_sb[:, :, :f], t2[:, :, :f])

    for u in range(UG):
        nc.sync.dma_start(out=outT[u * P:(u + 1) * P, base: base + f],
                          in_=h_sb[:, u, :f])
    if m_next > 0:
        nc.vector.tensor_copy(h_next[:, :, :m_next], h_sb[:, :, :m_next])


# ------------------------------------------------------------------- kernel

def kernel(x, h0, reset, W_ih, W_hh, b_ih, b_hh):
    global LAST_EXEC_NS
    x = np.asarray(x, np.float32)
    h0 = np.asarray(h0, np.float32)
    reset_sb = np.asarray(reset).reshape(SEQ, B).astype(bool)
    W_ih = np.asarray(W_ih, np.float32)
    W_hh = np.asarray(W_hh, np.float32)
    b_ih = np.asarray(b_ih, np.float32)
    b_hh = np.asarray(b_hh, np.float32)

    h0_any = bool(np.any(h0))
    m_j, plans = _build_plan(reset_sb, h0_any)
    N_pad = sum(m_j)

    b_sum = b_ih + b_hh
    biases = np.stack([b_sum[:UNITS], b_sum[UNITS:2 * UNITS],
                       b_ih[2 * UNITS:], WS * b_hh[2 * UNITS:]], axis=1)
    biases = np.ascontiguousarray(biases, np.float32)
    wihT = np.ascontiguousarray(W_ih.T * WS).astype(np.float16)
    whhnT = np.ascontiguousarray(W_hh[2 * UNITS:].T * WS).astype(np.float16)
    whh8T = np.ascontiguousarray(W_hh[:2 * UNITS].T * WS).astype(
        ml_dtypes.float8_e4m3)
    bnrow = (WS * np.stack([b_sum[:UNITS], b_sum[UNITS:2 * UNITS],
                            b_ih[2 * UNITS:], b_hh[2 * UNITS:]])
             ).astype(np.float16)
    bn8 = (WS * b_hh[2 * UNITS:]).reshape(UG, P).astype(np.float16)
    bsel = (np.arange(CH) // 64 == np.arange(UG)[:, None]).astype(np.float16)

    xf = x.reshape(SEQ * B, DIM)
    in_maps = []
    for c in range(NCORES):
        tok, seed_b = plans[c]
        real = tok >= 0
        xg = np.zeros((N_pad, DIM), np.float32)
        xg[real] = xf[tok[real]]
        m = {
            "xT": np.ascontiguousarray(xg.T).astype(np.float16),
            "wihT": wihT, "whhnT": whhnT, "whh8T": whh8T,
            "biases": biases, "bnrow": bnrow, "bn8": bn8, "bsel": bsel,
        }
        if h0_any:
            hs = np.zeros((m_j[0], UNITS), np.float32)
            sreal = seed_b >= 0
            hs[sreal] = h0[seed_b[sreal]]
            m["hseedT"] = np.ascontiguousarray(hs.T).astype(np.float16)
        in_maps.append(m)

    j_pre = 1
    while j_pre < len(m_j) and sum(m_j[j_pre:]) > CH:
        j_pre += 1
    j_tail = j_pre
    while j_tail < len(m_j) and m_j[j_tail] > 64:
        j_tail += 1
    if os.environ.get("GRU_TAILOFF", "0") == "1":
        j_tail = len(m_j)
    nc = _build_nc(m_j, use_seed=h0_any, j_pre=j_pre, j_tail=j_tail)
    trace = os.environ.get("GRU_TRACE", "0") == "1"
    res = run_bass_kernel_spmd(nc, in_maps, list(range(NCORES)), trace=trace)
    LAST_EXEC_NS = res.exec_time_ns

    out = np.zeros((SEQ * B, UNITS), np.float32)
    for c in range(NCORES):
        tok, _ = plans[c]
        real = tok >= 0
        out[tok[real]] = res.results[c]["outT"].T[real]
    return out.reshape(SEQ, B, UNITS)
